# revision 1
# baseline (speedup 1.0000x reference)
"""Grouped-Query Attention kernel for Trainium2 (8 NeuronCores, SPMD).

Problem: x [4, 4096, 512] fp32, per-group Dense Q/K/V (G=4 groups of 128
features), full softmax attention within each (batch, group) pair, output
re-concatenated to [4, 4096, 512].

Sharding: B*G = 16 fully independent attention problems -> 2 per core.
Per core, per pair, everything stays on-chip (SBUF 24MB):
  - load xg [4096, 128] fp32, cast bf16, PE-transpose to xgT [d, t]
  - Q^T = Wq^T xg^T, K^T likewise (bias folded in), V natural [t, e]
  - scores computed TRANSPOSED: S^T[ts, tq] = K Q^T so that the exp'd
    probabilities land directly in the layout attn@V needs as rhs
    (contraction dim ts on partitions) -- no transpose of the TxT matrix.
  - exp via ScalarE with the 1/sqrt(gs) scale folded into ACT's free affine
  - softmax denominator via an extra ones-matmul pass (all-rows-equal
    accumulate), out^T accumulated over ts chunks in PSUM
  - epilogue: reciprocal, normalize, +bv, PE-transpose back to natural
Compute dtype bf16 (fp32 accumulation in PSUM).
"""

import os
import sys

sys.path.insert(0, "/opt/trn_rl_repo")

import numpy as np

import concourse.bass as bass
import concourse.mybir as mybir
import concourse.tile as tile
from concourse.masks import make_identity

B, T, F, G = 4, 4096, 512, 4
GS = F // G  # 128
N_CORES = 8
PAIRS_PER_CORE = (B * G) // N_CORES  # 2
TQ_MACRO = 1024  # query tile width per softmax/psum round
N_MACROS = T // TQ_MACRO  # 4
N_CHUNKS = T // 128  # 32 key/time chunks
INV_SCALE = float(1.0 / (np.sqrt(np.float32(GS)) + 1e-9))

FP32 = mybir.dt.float32
BF16 = mybir.dt.bfloat16

_NC_CACHE = None
_LAST_IN_MAPS = None


def _split_multi_waits(nc):
    """Walrus codegen rejects instructions carrying more than one semaphore
    wait on several instruction structs (DMA DIRECT2D, tensor_scalar, LDW).
    Hoist all-but-the-last wait of any multi-wait instruction onto same-engine
    NoOps inserted immediately before it: the sequencer executes them in
    order, so the gating semantics are identical."""
    n_split = 0
    for func in nc.m.functions:
        for block in func.blocks:
            new = []
            for inst in block.instructions:
                si = inst.sync_info
                waits = list(si.on_wait) if (si is not None and si.on_wait) else []
                if len(waits) > 1:
                    for w in waits[:-1]:
                        nop = mybir.InstNoOp(
                            name=nc.get_next_instruction_name(), ins=[], outs=[]
                        )
                        nop.engine = inst.engine
                        nop.sync_info = mybir.SyncInfo(on_wait=[w], on_update=[])
                        new.append(nop)
                        n_split += 1
                    inst.sync_info = mybir.SyncInfo(
                        on_wait=[waits[-1]],
                        on_update=list(si.on_update) if si.on_update else [],
                    )
                new.append(inst)
            block.instructions = new
    return n_split


def build_nc():
    nc = bass.Bass()

    ins = []
    outs = []
    for i in range(PAIRS_PER_CORE):
        ins.append(
            dict(
                x=nc.declare_dram_parameter(f"x{i}", [T, GS], FP32, isOutput=False),
                wq=nc.declare_dram_parameter(f"wq{i}", [GS, GS], FP32, isOutput=False),
                wk=nc.declare_dram_parameter(f"wk{i}", [GS, GS], FP32, isOutput=False),
                wv=nc.declare_dram_parameter(f"wv{i}", [GS, GS], FP32, isOutput=False),
                bq=nc.declare_dram_parameter(f"bq{i}", [1, GS], FP32, isOutput=False),
                bk=nc.declare_dram_parameter(f"bk{i}", [1, GS], FP32, isOutput=False),
                bv=nc.declare_dram_parameter(f"bv{i}", [1, GS], FP32, isOutput=False),
            )
        )
        outs.append(nc.declare_dram_parameter(f"y{i}", [T, GS], FP32, isOutput=True))

    with tile.TileContext(nc) as tc:
        with (
            tc.tile_pool(name="consts", bufs=1) as consts,
            tc.tile_pool(name="bigsb", bufs=2) as bigsb,  # per-pair persistent
            tc.tile_pool(name="pt", bufs=4) as ptpool,  # exp'd prob chunks
            tc.tile_pool(name="epi", bufs=2) as epi,  # epilogue sbuf tiles
            tc.tile_pool(name="ps_s", bufs=2, space="PSUM") as ps_s,  # scores
            tc.tile_pool(name="ps_o", bufs=1, space="PSUM") as ps_o,  # out^T
            tc.tile_pool(name="ps_d", bufs=1, space="PSUM") as ps_d,  # denom
        ):
            ident_bf = consts.tile([128, 128], BF16)
            make_identity(nc, ident_bf)
            ident_f = consts.tile([128, 128], FP32)
            make_identity(nc, ident_f)
            ones_bf = consts.tile([128, 128], BF16)
            nc.vector.memset(ones_bf, 1.0)

            for i in range(PAIRS_PER_CORE):
                p = ins[i]
                # ---------------- prologue: load + QKV ----------------
                xg_f = bigsb.tile([128, N_CHUNKS, 128], FP32, tag="xg_f")
                nc.sync.dma_start(
                    out=xg_f, in_=p["x"][:, :].rearrange("(c p) d -> p c d", p=128)
                )
                xg_b = bigsb.tile([128, N_CHUNKS, 128], BF16, tag="xg_b")
                nc.vector.tensor_copy(xg_b, xg_f)

                # weights + biases
                w_bf = {}
                for nm in ("wq", "wk", "wv"):
                    wf = epi.tile([128, 128], FP32, tag=f"wf{nm}{i}")
                    nc.gpsimd.dma_start(out=wf, in_=p[nm][:, :])
                    wb = consts.tile([128, 128], BF16, tag=f"{nm}{i}")
                    nc.vector.tensor_copy(wb, wf)
                    w_bf[nm] = wb
                b_col = {}
                for nm in ("bq", "bk", "bv"):
                    bc = consts.tile([128, 1], FP32, tag=f"{nm}{i}")
                    nc.gpsimd.dma_start(
                        out=bc, in_=p[nm][:, :].rearrange("o d -> d o")
                    )
                    b_col[nm] = bc
                bvb = consts.tile([128, 128], FP32, tag=f"bvb{i}")
                _bv = p["bv"][:, :]
                nc.gpsimd.dma_start(
                    out=bvb,
                    in_=bass.AP(tensor=_bv.tensor, offset=_bv.offset,
                                ap=[[0, 128]] + list(_bv.ap[1:])),
                )

                # xgT [d, t] bf16 via PE transpose of 32 chunks
                xgT = bigsb.tile([128, T], BF16, tag="xgT")
                for c in range(N_CHUNKS):
                    pst = ps_s.tile([128, 128], BF16, tag="sc")
                    nc.tensor.transpose(pst, xg_b[:, c, :], ident_bf)
                    nc.vector.tensor_copy(xgT[:, c * 128 : (c + 1) * 128], pst)

                # Q^T/K^T [e, t] bf16 (bias added), V^T -> V natural
                qt = bigsb.tile([128, T], BF16, tag="qt")
                kt = bigsb.tile([128, T], BF16, tag="kt")
                vt = bigsb.tile([128, T], BF16, tag="vt")
                for dst, wname, bname in (
                    (qt, "wq", "bq"),
                    (kt, "wk", "bk"),
                    (vt, "wv", None),
                ):
                    for j in range(T // TQ_MACRO):
                        psq = ps_s.tile([128, TQ_MACRO], FP32, tag="sc")
                        for h in range(TQ_MACRO // 512):
                            sl = slice(h * 512, (h + 1) * 512)
                            tsl = slice(j * TQ_MACRO + h * 512, j * TQ_MACRO + (h + 1) * 512)
                            nc.tensor.matmul(
                                psq[:, sl], w_bf[wname], xgT[:, tsl], start=True, stop=True
                            )
                        dsl = slice(j * TQ_MACRO, (j + 1) * TQ_MACRO)
                        if bname is not None:
                            nc.vector.tensor_scalar_add(dst[:, dsl], psq, b_col[bname])
                        else:
                            nc.vector.tensor_copy(dst[:, dsl], psq)

                v_nat = bigsb.tile([128, N_CHUNKS, 128], BF16, tag="v_nat")
                for c in range(N_CHUNKS):
                    pst = ps_s.tile([128, 128], BF16, tag="sc")
                    nc.tensor.transpose(pst, vt[:, c * 128 : (c + 1) * 128], ident_bf)
                    nc.vector.tensor_copy(v_nat[:, c, :], pst)

                # ---------------- attention macros ----------------
                for m in range(N_MACROS):
                    tq0 = m * TQ_MACRO
                    ps_out = ps_o.tile([128, TQ_MACRO], FP32)
                    ps_den = ps_d.tile([128, TQ_MACRO], FP32)
                    for c in range(N_CHUNKS):
                        ksl = kt[:, c * 128 : (c + 1) * 128]
                        ps_sc = ps_s.tile([128, TQ_MACRO], FP32, tag="sc")
                        for h in range(TQ_MACRO // 512):
                            sl = slice(h * 512, (h + 1) * 512)
                            qsl = slice(tq0 + h * 512, tq0 + (h + 1) * 512)
                            nc.tensor.matmul(
                                ps_sc[:, sl], ksl, qt[:, qsl], start=True, stop=True
                            )
                        pt = ptpool.tile([128, TQ_MACRO], BF16)
                        nc.scalar.activation(
                            pt, ps_sc, mybir.ActivationFunctionType.Exp, scale=INV_SCALE
                        )
                        first, last = c == 0, c == N_CHUNKS - 1
                        for h in range(TQ_MACRO // 512):
                            sl = slice(h * 512, (h + 1) * 512)
                            nc.tensor.matmul(
                                ps_out[:, sl], v_nat[:, c, :], pt[:, sl],
                                start=first, stop=last,
                            )
                            nc.tensor.matmul(
                                ps_den[:, sl], ones_bf, pt[:, sl],
                                start=first, stop=last,
                            )
                    recip = epi.tile([128, TQ_MACRO], FP32, tag="recip")
                    nc.vector.reciprocal(recip, ps_den)
                    onorm = epi.tile([128, TQ_MACRO], FP32, tag="onorm")
                    nc.vector.tensor_mul(onorm, ps_out, recip)
                    nc.vector.tensor_scalar_add(onorm, onorm, b_col["bv"])
                    onat = epi.tile([128, TQ_MACRO // 128, 128], FP32, tag="onat")
                    for j in range(TQ_MACRO // 128):
                        pst = ps_s.tile([128, 128], FP32, tag="sc")
                        nc.tensor.transpose(pst, onorm[:, j * 128 : (j + 1) * 128], ident_f)
                        nc.vector.tensor_copy(onat[:, j, :], pst)
                    nc.sync.dma_start(
                        out=outs[i][tq0 : tq0 + TQ_MACRO, :].rearrange(
                            "(c p) d -> p c d", p=128
                        ),
                        in_=onat,
                    )
    _split_multi_waits(nc)
    return nc


def _get_nc():
    global _NC_CACHE
    if _NC_CACHE is None:
        _NC_CACHE = build_nc()
    return _NC_CACHE


def kernel(**inputs: np.ndarray) -> np.ndarray:
    x = np.ascontiguousarray(inputs["x"], dtype=np.float32)
    Wq = np.asarray(inputs["Wq"], dtype=np.float32)
    Wk = np.asarray(inputs["Wk"], dtype=np.float32)
    Wv = np.asarray(inputs["Wv"], dtype=np.float32)
    bq = np.asarray(inputs["bq"], dtype=np.float32)
    bk = np.asarray(inputs["bk"], dtype=np.float32)
    bv = np.asarray(inputs["bv"], dtype=np.float32)

    nc = _get_nc()

    in_maps = []
    for core in range(N_CORES):
        m = {}
        for i in range(PAIRS_PER_CORE):
            pair = core * PAIRS_PER_CORE + i
            b, g = pair // G, pair % G
            sl = slice(g * GS, (g + 1) * GS)
            m[f"x{i}"] = np.ascontiguousarray(x[b, :, sl])
            m[f"wq{i}"] = np.ascontiguousarray(Wq[g])
            m[f"wk{i}"] = np.ascontiguousarray(Wk[g])
            m[f"wv{i}"] = np.ascontiguousarray(Wv[g])
            m[f"bq{i}"] = np.ascontiguousarray(bq[g].reshape(1, GS))
            m[f"bk{i}"] = np.ascontiguousarray(bk[g].reshape(1, GS))
            m[f"bv{i}"] = np.ascontiguousarray(bv[g].reshape(1, GS))
        in_maps.append(m)

    global _LAST_IN_MAPS
    _LAST_IN_MAPS = in_maps

    from concourse.bass_utils import run_bass_kernel_spmd

    res = run_bass_kernel_spmd(nc, in_maps, list(range(N_CORES)))

    y = np.empty((B, T, F), dtype=np.float32)
    for core in range(N_CORES):
        for i in range(PAIRS_PER_CORE):
            pair = core * PAIRS_PER_CORE + i
            b, g = pair // G, pair % G
            y[b, :, g * GS : (g + 1) * GS] = res.results[core][f"y{i}"]
    return y

